# revision 22
# baseline (speedup 1.0000x reference)
"""TuckER scoring kernel for 8 Trainium2 NeuronCores.

Model: e1 = E1[X[:,0]]; r = R[X[:,1]]
       x[b,k] = sum_{i,j} r[b,i] * e1[b,j] * W[i,j,k]
       out    = sigmoid(x @ E2.T)            # [B, N_ENT]

Structure:
  - stage 1 (x, a [512, 200] matrix) is tiny: one host sgemm
    z = r @ W.reshape(D, D*D) plus a 20M-element contraction with e1.
    It is computed on host (like the baseline's host-side Khatri-Rao
    lift), pre-scaled by S, and uploaded as x.T in fp8-e4m3.
  - stage 2 is tensor-parallel over the entity vocab: core m owns E2
    rows [12500m, 12500(m+1)) and computes S*logits = (S*x) @ E2_m.T
    as an fp8-e4m3 DoubleRow matmul (K=200 padded to 256 = 128x2, one
    PE pass per 500-wide tile at 2 MACs/cell/cycle) with fp32 PSUM
    accumulation.
  - PSUM fp32 -> int8 conversion (alternating DVE / ACT so neither
    engine is the bottleneck), int8 shipped to DRAM in bc-pair merged
    512KB DMAs: the output stream is a quarter the bytes of fp16, and
    sigmoid collapses to a 256-entry host lookup table applied to the
    int8 logits.
  - DMA issue placement matters: e2 chunk loads must NOT sit in the
    convert engines' (scalar/vector) instruction queues or their ring
    backpressure stalls the converts (and then PSUM, and then the PE).
    Inputs ride sync (first chunks) + gpsimd/SWDGE (rest); outputs
    ride sync.
No collectives; nothing device-side depends on another core.
"""

import numpy as np
import ml_dtypes

N_ENT = 100000
N_REL = 500
D = 200
B = 512
NC = 8
NSH = N_ENT // NC       # 12500 entity rows per core
KP = 128                # contraction partition rows
KO = 2                  # DoubleRow k-tiles per partition row
KPAD = KP * KO          # 256 (200 zero-padded)
NT = 512                # logits matmul free-dim tile = one full PSUM bank,
                        # so PSUM groups are contiguous (single-run converts)
SLOTW = 1024            # PSUM ring slot width (2 banks); 4 slots in flight
# cols per chunk of the combined [x.T | E2.T] tensor: chunk 0 is x.T
# alone (small, lands first: it gates the warm-up matmuls and the first
# LDWEIGHTS). Few, large chunks after that: each gpsimd SWDGE dma_start
# costs ~2.5us of serialized Q7 descriptor emission, and late chunks
# starve the matmul stream (the straggler-core signature).
CHUNK_COLS = [B, 1024, 1024, 2048, 2048, 4096, 2260]
SYNC_CHUNKS = 5         # chunks loaded via the sync ring (before outputs)
WARM_MM = 4             # dummy matmuls ahead of the real stream: together
                        # with the first (cold) real matmuls they span the
                        # HAM activity window, so the bulk runs at 2.4GHz
SCALE = 112.0           # int8 logit scale; max |logit| ~= 1.08 -> |q| <= 121

_BF16 = ml_dtypes.bfloat16
_FP8 = ml_dtypes.float8_e4m3

_cached = {}


def _build_bass():
    from contextlib import ExitStack
    import concourse.tile as tile
    from concourse import bacc, mybir

    f32 = mybir.dt.float32
    fp8 = mybir.dt.float8e4
    i8 = mybir.dt.int8

    nc = bacc.Bacc("TRN2", target_bir_lowering=False, debug=False,
                   num_devices=NC)
    et_d = nc.declare_dram_parameter("et", [KPAD, B + NSH], fp8,
                                     isOutput=False)
    out_d = nc.declare_dram_parameter("out", [B, NSH], i8, isOutput=True)

    # [kp, ko, *]: contraction row k = ko*128 + kp (DoubleRow pairing)
    et_v = et_d.rearrange("(ko kp) n -> kp ko n", kp=KP)
    # [p, g, n]: output row = g*128 + p (bc-pair merged output DMAs)
    out_v = out_d.rearrange("(g p) n -> p g n", p=128)

    with tile.TileContext(nc) as tc, ExitStack() as ctx:
        ipool = ctx.enter_context(tc.tile_pool(name="inp", bufs=1))
        opool = ctx.enter_context(tc.tile_pool(name="outp", bufs=6))

        # Chunked loads of [x.T | E2.T]: first chunks on sync, the rest on
        # the otherwise-idle gpsimd SWDGE. Chunk 0 also carries x.T.
        e2c, c0s = [], []
        c0 = 0
        for ci, w in enumerate(CHUNK_COLS):
            cw = ((w + 15) // 16) * 16   # ko-stride must be 16B-aligned
            c = ipool.tile([KP, KO, cw], fp8, tag=f"e2c{ci}")
            eng = nc.sync if ci < SYNC_CHUNKS else nc.gpsimd
            eng.dma_start(c[:, :, 0:w], et_v[:, :, c0:c0 + w])
            e2c.append(c)
            c0s.append(c0 - B)     # E2-space column of this chunk's col 0
            c0 += w
        x8 = e2c[0]                # x.T = cols [0, B) of chunk 0

        # PE warm-up: dummy matmuls on x.T (head chunk only) that run as
        # one dense stream with the real matmuls — they hold the PE busy
        # through the HAM activity window (~3.4us), so the real stream
        # runs at 2.4GHz from its first tile instead of 1.2.

        # stage 2: one DoubleRow matmul per 512-wide n-tile (full K in a
        # single pass). PSUM is managed as a manual 4-slot ring of 1024-col
        # (2-bank) regions inside one tile — Tile's byte-range dependency
        # tracking gives 4 groups in flight, so the convert latency is off
        # the MM critical path (vs. 2 with a bufs=2 pool of 4-bank tiles).
        # Convert fp32 PSUM -> int8 on DVE/ACT (greedy load-balance);
        # merge each column range's 4 batch chunks into one ~512KB output
        # DMA on the sync ring.
        eng_busy = {"v": 0.0, "s": 0.0}   # projected convert busy (ns)
        gi = 0                            # psum ring position
        with tc.tile_pool(name="ps", bufs=1, space="PSUM") as ps:
            pall = ps.tile([128, 4096], f32, tag="pall")
            for _ in range(WARM_MM):
                nc.tensor.matmul(
                    pall[:, 0:512], x8[:, :, 0:128], x8[:, :, 0:B],
                    start=True, stop=True,
                    perf_mode=mybir.MatmulPerfMode.DoubleRow)
            for ci, w in enumerate(CHUNK_COLS):
                for s0 in range(B if ci == 0 else 0, w, SLOTW):
                    sw = min(SLOTW, w - s0)
                    otq = opool.tile([128, 4, SLOTW], i8,
                                     name="otq", tag="otq")
                    for bc in range(4):
                        slot = (gi % 4) * SLOTW
                        gi += 1
                        for t0 in range(s0, s0 + sw, NT):
                            tw = min(NT, s0 + sw - t0)
                            nc.tensor.matmul(
                                pall[:, slot + t0 - s0:slot + t0 - s0 + tw],
                                x8[:, :, bc * 128:(bc + 1) * 128],
                                e2c[ci][:, :, t0:t0 + tw],
                                start=True, stop=True,
                                perf_mode=mybir.MatmulPerfMode.DoubleRow)
                        cost_v = (sw * 1.26 + 165)
                        cost_s = (sw * 1.01 + 293)
                        if eng_busy["v"] + cost_v <= eng_busy["s"] + cost_s:
                            nc.vector.tensor_copy(
                                otq[:, bc, 0:sw], pall[:, slot:slot + sw])
                            eng_busy["v"] += cost_v
                        else:
                            nc.scalar.copy(
                                otq[:, bc, 0:sw], pall[:, slot:slot + sw])
                            eng_busy["s"] += cost_s
                    # the very last quad rides the scalar ring (nothing is
                    # queued behind ACT's final convert) so the two tail
                    # DMAs drain in parallel
                    last = (ci == len(CHUNK_COLS) - 1 and s0 + sw >= w)
                    (nc.scalar if last else nc.sync).dma_start(
                        out_v[:, :, c0s[ci] + s0:c0s[ci] + s0 + sw],
                        otq[:, :, 0:sw])

    nc.compile()
    return nc


def _prep_in_maps(X, E1, R, E2, W):
    X = np.asarray(X)
    E1 = np.asarray(E1, dtype=np.float32)
    R = np.asarray(R, dtype=np.float32)
    E2 = np.asarray(E2, dtype=np.float32)
    W = np.asarray(W, dtype=np.float32)

    e1 = E1[np.asarray(X[:, 0], dtype=np.int64)]   # [B, D]
    r = R[np.asarray(X[:, 1], dtype=np.int64)]     # [B, D]

    # x[b,k] = sum_{i,j} r[b,i] e1[b,j] W[i,j,k]  (one sgemm + a small
    # batched contraction), pre-scaled so PSUM holds SCALE * logits.
    z = r @ W.reshape(D, D * D)                    # [B, D*D]
    x = np.einsum('bjk,bj->bk', z.reshape(B, D, D), e1,
                  optimize=True)                   # [B, D]
    xt = np.ascontiguousarray((x * SCALE).T).astype(_FP8)   # [D, B]

    in_maps = []
    for m in range(NC):
        nsl = slice(m * NSH, (m + 1) * NSH)
        et = np.zeros((KPAD, B + NSH), dtype=_FP8)
        et[:D, :B] = xt
        et[:D, B:] = np.ascontiguousarray(E2[nsl].T).astype(_FP8)
        in_maps.append({"et": et})
    return in_maps


def _postprocess(res):
    """int8 logits -> sigmoid via a 256-entry LUT, concat over cores."""
    if "lut" not in _cached:
        u = np.arange(256, dtype=np.int64)
        signed = np.where(u < 128, u, u - 256).astype(np.float64)
        _cached["lut"] = (1.0 / (1.0 + np.exp(-signed / SCALE))).astype(
            np.float32)
    lut = _cached["lut"]
    q = np.concatenate([res[m]["out"] for m in range(NC)], axis=1)
    return lut[q.view(np.uint8)]


def _get_nc():
    if "nc" not in _cached:
        _cached["nc"] = _build_bass()
    return _cached["nc"]


def _get_exec():
    """Build (once) a cached jit-compiled SPMD executable for the Bass module.

    Mirrors concourse.bass2jax.run_bass_via_pjrt, but hoists the jit callable
    into a module-level cache so repeated kernel() calls don't recompile.
    """
    if "exec" in _cached:
        return _cached["exec"]

    import jax
    import numpy as _np
    from jax.sharding import Mesh, PartitionSpec
    from jax.experimental.shard_map import shard_map
    from concourse import mybir
    from concourse.bass2jax import (
        install_neuronx_cc_hook, _bass_exec_p, partition_id_tensor)

    nc = _get_nc()
    install_neuronx_cc_hook()

    partition_name = (
        nc.partition_id_tensor.name if nc.partition_id_tensor else None)
    in_names, out_names, out_avals, zero_outs = [], [], [], []
    for alloc in nc.m.functions[0].allocations:
        if not isinstance(alloc, mybir.MemoryLocationSet):
            continue
        name = alloc.memorylocations[0].name
        if alloc.kind == "ExternalInput":
            if name != partition_name:
                in_names.append(name)
        elif alloc.kind == "ExternalOutput":
            out_names.append(name)
            shape = tuple(alloc.tensor_shape)
            dtype = mybir.dt.np(alloc.dtype)
            out_avals.append(jax.core.ShapedArray(shape, dtype))
            zero_outs.append(_np.zeros(shape, dtype))
    n_params = len(in_names)
    n_outs = len(out_avals)
    all_in_names = list(in_names) + list(out_names)
    if partition_name is not None:
        all_in_names.append(partition_name)
    donate = tuple(range(n_params, n_params + n_outs))

    def _body(*args):
        operands = list(args)
        if partition_name is not None:
            operands.append(partition_id_tensor())
        outs = _bass_exec_p.bind(
            *operands,
            out_avals=tuple(out_avals),
            in_names=tuple(all_in_names),
            out_names=tuple(out_names),
            lowering_input_output_aliases=(),
            sim_require_finite=True,
            sim_require_nnan=True,
            nc=nc,
        )
        return tuple(outs)

    devices = jax.devices()[:NC]
    mesh = Mesh(np.asarray(devices), ("core",))
    in_specs = (PartitionSpec("core"),) * (n_params + n_outs)
    out_specs = (PartitionSpec("core"),) * n_outs
    sharded = jax.jit(
        shard_map(_body, mesh=mesh, in_specs=in_specs, out_specs=out_specs,
                  check_rep=False),
        donate_argnums=donate, keep_unused=True)
    _cached["exec"] = (sharded, in_names, out_names, out_avals, zero_outs)
    return _cached["exec"]


def _upload_inputs(in_maps):
    """Transfer per-core inputs to the devices once; returns device arrays
    shardable by the cached executable (inputs are not donated, so they can
    be reused across executions without re-uploading)."""
    import jax
    from jax.sharding import Mesh, PartitionSpec, NamedSharding
    sharded, in_names, out_names, out_avals, zero_outs = _get_exec()
    n = len(in_maps)
    devices = jax.devices()[:NC]
    mesh = Mesh(np.asarray(devices), ("core",))
    sh = NamedSharding(mesh, PartitionSpec("core"))
    dev_in = [
        jax.device_put(
            np.concatenate([np.asarray(in_maps[c][name]) for c in range(n)],
                           axis=0), sh)
        for name in in_names]
    for a in dev_in:
        a.block_until_ready()
    return dev_in


def _exec_once(dev_in):
    """One device execution using already-uploaded inputs."""
    import jax
    import jax.numpy as jnp
    from jax.sharding import Mesh, PartitionSpec, NamedSharding
    sharded, in_names, out_names, out_avals, zero_outs = _get_exec()
    n = NC
    if "zeros_fn" not in _cached:
        devices = jax.devices()[:NC]
        mesh = Mesh(np.asarray(devices), ("core",))
        sh = NamedSharding(mesh, PartitionSpec("core"))
        shapes = [((n * z.shape[0], *z.shape[1:]), z.dtype) for z in zero_outs]
        _cached["zeros_fn"] = jax.jit(
            lambda: tuple(jnp.zeros(s, d) for s, d in shapes),
            out_shardings=tuple(sh for _ in shapes))
    concat_zeros = list(_cached["zeros_fn"]())
    out_arrs = sharded(*dev_in, *concat_zeros)
    for a in out_arrs:
        a.block_until_ready()
    return out_arrs


def _collect(out_arrs):
    _, in_names, out_names, out_avals, _ = _get_exec()
    return [
        {name: np.asarray(out_arrs[i]).reshape(NC, *out_avals[i].shape)[c]
         for i, name in enumerate(out_names)}
        for c in range(NC)]


def kernel(X, E1, R, E2, W):
    in_maps = _prep_in_maps(X, E1, R, E2, W)
    dev_in = _upload_inputs(in_maps)
    if "warm" not in _cached:
        # first call: run once so the NEFF is loaded on every core before
        # the "real" execution (cold NEFF loads stagger core start times
        # and inflate cross-core sync waits)
        _exec_once(dev_in)
        _cached["warm"] = True
    res = _collect(_exec_once(dev_in))
    return _postprocess(res)


# revision 23
# speedup vs baseline: 1.0248x; 1.0248x over previous
"""TuckER scoring kernel for 8 Trainium2 NeuronCores.

Model: e1 = E1[X[:,0]]; r = R[X[:,1]]
       x[b,k] = sum_{i,j} r[b,i] * e1[b,j] * W[i,j,k]
       out    = sigmoid(x @ E2.T)            # [B, N_ENT]

Structure:
  - stage 1 (x, a [512, 200] matrix) is tiny: one host sgemm
    z = r @ W.reshape(D, D*D) plus a 20M-element contraction with e1.
    It is computed on host (like the baseline's host-side Khatri-Rao
    lift), pre-scaled by S, and uploaded as x.T in fp8-e4m3.
  - stage 2 is tensor-parallel over the entity vocab: core m owns E2
    rows [12500m, 12500(m+1)) and computes S*logits = (S*x) @ E2_m.T
    as an fp8-e4m3 DoubleRow matmul (K=200 padded to 256 = 128x2, one
    PE pass per 500-wide tile at 2 MACs/cell/cycle) with fp32 PSUM
    accumulation.
  - PSUM fp32 -> int8 conversion (alternating DVE / ACT so neither
    engine is the bottleneck), int8 shipped to DRAM in bc-pair merged
    512KB DMAs: the output stream is a quarter the bytes of fp16, and
    sigmoid collapses to a 256-entry host lookup table applied to the
    int8 logits.
  - DMA issue placement matters: e2 chunk loads must NOT sit in the
    convert engines' (scalar/vector) instruction queues or their ring
    backpressure stalls the converts (and then PSUM, and then the PE).
    Inputs ride sync (first chunks) + gpsimd/SWDGE (rest); outputs
    ride sync.
No collectives; nothing device-side depends on another core.
"""

import numpy as np
import ml_dtypes

N_ENT = 100000
N_REL = 500
D = 200
B = 512
NC = 8
NSH = N_ENT // NC       # 12500 entity rows per core
KP = 128                # contraction partition rows
KO = 2                  # DoubleRow k-tiles per partition row
KPAD = KP * KO          # 256 (200 zero-padded)
NT = 512                # logits matmul free-dim tile = one full PSUM bank,
                        # so PSUM groups are contiguous (single-run converts)
SLOTW = 1024            # PSUM ring slot width (2 banks); 4 slots in flight
# cols per chunk of the combined [x.T | E2.T] tensor: chunk 0 carries x.T
# plus the first 512 E2 cols in ONE DMA (one semaphore gates the warm-up
# matmuls, the first LDWEIGHTS, AND the first real matmul — separate
# head DMAs leave a PE gap that resets the HAM window). Few, large
# chunks: each gpsimd SWDGE dma_start costs ~2.5us of serialized Q7
# descriptor emission, and late chunks starve the matmul stream (the
# straggler-core signature).
CHUNK_COLS = [B + 512, 1024, 2048, 2048, 4096, 2772]
SYNC_CHUNKS = 4         # chunks loaded via the sync ring (before outputs)
WARM_MM = 8             # dummy matmuls to hold PE busy through the HAM
                        # activity window so the real stream starts warm
SCALE = 112.0           # int8 logit scale; max |logit| ~= 1.08 -> |q| <= 121

_BF16 = ml_dtypes.bfloat16
_FP8 = ml_dtypes.float8_e4m3

_cached = {}


def _build_bass():
    from contextlib import ExitStack
    import concourse.tile as tile
    from concourse import bacc, mybir

    f32 = mybir.dt.float32
    fp8 = mybir.dt.float8e4
    i8 = mybir.dt.int8

    nc = bacc.Bacc("TRN2", target_bir_lowering=False, debug=False,
                   num_devices=NC)
    et_d = nc.declare_dram_parameter("et", [KPAD, B + NSH], fp8,
                                     isOutput=False)
    out_d = nc.declare_dram_parameter("out", [B, NSH], i8, isOutput=True)

    # [kp, ko, *]: contraction row k = ko*128 + kp (DoubleRow pairing)
    et_v = et_d.rearrange("(ko kp) n -> kp ko n", kp=KP)
    # [p, g, n]: output row = g*128 + p (bc-pair merged output DMAs)
    out_v = out_d.rearrange("(g p) n -> p g n", p=128)

    with tile.TileContext(nc) as tc, ExitStack() as ctx:
        ipool = ctx.enter_context(tc.tile_pool(name="inp", bufs=1))
        opool = ctx.enter_context(tc.tile_pool(name="outp", bufs=6))

        # Chunked loads of [x.T | E2.T]: first chunks on sync, the rest on
        # the otherwise-idle gpsimd SWDGE. Chunk 0 also carries x.T.
        e2c, c0s = [], []
        c0 = 0
        for ci, w in enumerate(CHUNK_COLS):
            cw = ((w + 15) // 16) * 16   # ko-stride must be 16B-aligned
            c = ipool.tile([KP, KO, cw], fp8, tag=f"e2c{ci}")
            eng = nc.sync if ci < SYNC_CHUNKS else nc.gpsimd
            eng.dma_start(c[:, :, 0:w], et_v[:, :, c0:c0 + w])
            e2c.append(c)
            c0s.append(c0 - B)     # E2-space column of this chunk's col 0
            c0 += w
        x8 = e2c[0]                # x.T = cols [0, B) of chunk 0

        # PE warm-up: dummy matmuls on x.T (head chunk only) that run as
        # one dense stream with the real matmuls — they hold the PE busy
        # through the HAM activity window (~3.4us), so the real stream
        # runs at 2.4GHz from its first tile instead of 1.2.

        # stage 2: one DoubleRow matmul per 512-wide n-tile (full K in a
        # single pass). PSUM is managed as a manual 4-slot ring of 1024-col
        # (2-bank) regions inside one tile — Tile's byte-range dependency
        # tracking gives 4 groups in flight, so the convert latency is off
        # the MM critical path (vs. 2 with a bufs=2 pool of 4-bank tiles).
        # Convert fp32 PSUM -> int8 on DVE/ACT (greedy load-balance);
        # merge each column range's 4 batch chunks into one ~512KB output
        # DMA on the sync ring.
        eng_busy = {"v": 0.0, "s": 0.0}   # projected convert busy (ns)
        gi = 0                            # psum ring position
        with tc.tile_pool(name="ps", bufs=1, space="PSUM") as ps:
            pall = ps.tile([128, 4096], f32, tag="pall")
            for _ in range(WARM_MM):
                nc.tensor.matmul(
                    pall[:, 0:512], x8[:, :, 0:128], x8[:, :, 0:B],
                    start=True, stop=True,
                    perf_mode=mybir.MatmulPerfMode.DoubleRow)
            for ci, w in enumerate(CHUNK_COLS):
                for s0 in range(B if ci == 0 else 0, w, SLOTW):
                    sw = min(SLOTW, w - s0)
                    otq = opool.tile([128, 4, SLOTW], i8,
                                     name="otq", tag="otq")
                    for bc in range(4):
                        slot = (gi % 4) * SLOTW
                        gi += 1
                        for t0 in range(s0, s0 + sw, NT):
                            tw = min(NT, s0 + sw - t0)
                            nc.tensor.matmul(
                                pall[:, slot + t0 - s0:slot + t0 - s0 + tw],
                                x8[:, :, bc * 128:(bc + 1) * 128],
                                e2c[ci][:, :, t0:t0 + tw],
                                start=True, stop=True,
                                perf_mode=mybir.MatmulPerfMode.DoubleRow)
                        cost_v = (sw * 1.26 + 165)
                        cost_s = (sw * 1.01 + 293)
                        if eng_busy["v"] + cost_v <= eng_busy["s"] + cost_s:
                            nc.vector.tensor_copy(
                                otq[:, bc, 0:sw], pall[:, slot:slot + sw])
                            eng_busy["v"] += cost_v
                        else:
                            nc.scalar.copy(
                                otq[:, bc, 0:sw], pall[:, slot:slot + sw])
                            eng_busy["s"] += cost_s
                    # the very last quad rides the scalar ring (nothing is
                    # queued behind ACT's final convert) so the two tail
                    # DMAs drain in parallel
                    last = (ci == len(CHUNK_COLS) - 1 and s0 + sw >= w)
                    (nc.scalar if last else nc.sync).dma_start(
                        out_v[:, :, c0s[ci] + s0:c0s[ci] + s0 + sw],
                        otq[:, :, 0:sw])

    nc.compile()
    return nc


def _prep_in_maps(X, E1, R, E2, W):
    X = np.asarray(X)
    E1 = np.asarray(E1, dtype=np.float32)
    R = np.asarray(R, dtype=np.float32)
    E2 = np.asarray(E2, dtype=np.float32)
    W = np.asarray(W, dtype=np.float32)

    e1 = E1[np.asarray(X[:, 0], dtype=np.int64)]   # [B, D]
    r = R[np.asarray(X[:, 1], dtype=np.int64)]     # [B, D]

    # x[b,k] = sum_{i,j} r[b,i] e1[b,j] W[i,j,k]  (one sgemm + a small
    # batched contraction), pre-scaled so PSUM holds SCALE * logits.
    z = r @ W.reshape(D, D * D)                    # [B, D*D]
    x = np.einsum('bjk,bj->bk', z.reshape(B, D, D), e1,
                  optimize=True)                   # [B, D]
    xt = np.ascontiguousarray((x * SCALE).T).astype(_FP8)   # [D, B]

    in_maps = []
    for m in range(NC):
        nsl = slice(m * NSH, (m + 1) * NSH)
        et = np.zeros((KPAD, B + NSH), dtype=_FP8)
        et[:D, :B] = xt
        et[:D, B:] = np.ascontiguousarray(E2[nsl].T).astype(_FP8)
        in_maps.append({"et": et})
    return in_maps


def _postprocess(res):
    """int8 logits -> sigmoid via a 256-entry LUT, concat over cores."""
    if "lut" not in _cached:
        u = np.arange(256, dtype=np.int64)
        signed = np.where(u < 128, u, u - 256).astype(np.float64)
        _cached["lut"] = (1.0 / (1.0 + np.exp(-signed / SCALE))).astype(
            np.float32)
    lut = _cached["lut"]
    q = np.concatenate([res[m]["out"] for m in range(NC)], axis=1)
    return lut[q.view(np.uint8)]


def _get_nc():
    if "nc" not in _cached:
        _cached["nc"] = _build_bass()
    return _cached["nc"]


def _get_exec():
    """Build (once) a cached jit-compiled SPMD executable for the Bass module.

    Mirrors concourse.bass2jax.run_bass_via_pjrt, but hoists the jit callable
    into a module-level cache so repeated kernel() calls don't recompile.
    """
    if "exec" in _cached:
        return _cached["exec"]

    import jax
    import numpy as _np
    from jax.sharding import Mesh, PartitionSpec
    from jax.experimental.shard_map import shard_map
    from concourse import mybir
    from concourse.bass2jax import (
        install_neuronx_cc_hook, _bass_exec_p, partition_id_tensor)

    nc = _get_nc()
    install_neuronx_cc_hook()

    partition_name = (
        nc.partition_id_tensor.name if nc.partition_id_tensor else None)
    in_names, out_names, out_avals, zero_outs = [], [], [], []
    for alloc in nc.m.functions[0].allocations:
        if not isinstance(alloc, mybir.MemoryLocationSet):
            continue
        name = alloc.memorylocations[0].name
        if alloc.kind == "ExternalInput":
            if name != partition_name:
                in_names.append(name)
        elif alloc.kind == "ExternalOutput":
            out_names.append(name)
            shape = tuple(alloc.tensor_shape)
            dtype = mybir.dt.np(alloc.dtype)
            out_avals.append(jax.core.ShapedArray(shape, dtype))
            zero_outs.append(_np.zeros(shape, dtype))
    n_params = len(in_names)
    n_outs = len(out_avals)
    all_in_names = list(in_names) + list(out_names)
    if partition_name is not None:
        all_in_names.append(partition_name)
    donate = tuple(range(n_params, n_params + n_outs))

    def _body(*args):
        operands = list(args)
        if partition_name is not None:
            operands.append(partition_id_tensor())
        outs = _bass_exec_p.bind(
            *operands,
            out_avals=tuple(out_avals),
            in_names=tuple(all_in_names),
            out_names=tuple(out_names),
            lowering_input_output_aliases=(),
            sim_require_finite=True,
            sim_require_nnan=True,
            nc=nc,
        )
        return tuple(outs)

    devices = jax.devices()[:NC]
    mesh = Mesh(np.asarray(devices), ("core",))
    in_specs = (PartitionSpec("core"),) * (n_params + n_outs)
    out_specs = (PartitionSpec("core"),) * n_outs
    sharded = jax.jit(
        shard_map(_body, mesh=mesh, in_specs=in_specs, out_specs=out_specs,
                  check_rep=False),
        donate_argnums=donate, keep_unused=True)
    _cached["exec"] = (sharded, in_names, out_names, out_avals, zero_outs)
    return _cached["exec"]


def _upload_inputs(in_maps):
    """Transfer per-core inputs to the devices once; returns device arrays
    shardable by the cached executable (inputs are not donated, so they can
    be reused across executions without re-uploading)."""
    import jax
    from jax.sharding import Mesh, PartitionSpec, NamedSharding
    sharded, in_names, out_names, out_avals, zero_outs = _get_exec()
    n = len(in_maps)
    devices = jax.devices()[:NC]
    mesh = Mesh(np.asarray(devices), ("core",))
    sh = NamedSharding(mesh, PartitionSpec("core"))
    dev_in = [
        jax.device_put(
            np.concatenate([np.asarray(in_maps[c][name]) for c in range(n)],
                           axis=0), sh)
        for name in in_names]
    for a in dev_in:
        a.block_until_ready()
    return dev_in


def _exec_once(dev_in):
    """One device execution using already-uploaded inputs."""
    import jax
    import jax.numpy as jnp
    from jax.sharding import Mesh, PartitionSpec, NamedSharding
    sharded, in_names, out_names, out_avals, zero_outs = _get_exec()
    n = NC
    if "zeros_fn" not in _cached:
        devices = jax.devices()[:NC]
        mesh = Mesh(np.asarray(devices), ("core",))
        sh = NamedSharding(mesh, PartitionSpec("core"))
        shapes = [((n * z.shape[0], *z.shape[1:]), z.dtype) for z in zero_outs]
        _cached["zeros_fn"] = jax.jit(
            lambda: tuple(jnp.zeros(s, d) for s, d in shapes),
            out_shardings=tuple(sh for _ in shapes))
    concat_zeros = list(_cached["zeros_fn"]())
    out_arrs = sharded(*dev_in, *concat_zeros)
    for a in out_arrs:
        a.block_until_ready()
    return out_arrs


def _collect(out_arrs):
    _, in_names, out_names, out_avals, _ = _get_exec()
    return [
        {name: np.asarray(out_arrs[i]).reshape(NC, *out_avals[i].shape)[c]
         for i, name in enumerate(out_names)}
        for c in range(NC)]


def kernel(X, E1, R, E2, W):
    in_maps = _prep_in_maps(X, E1, R, E2, W)
    dev_in = _upload_inputs(in_maps)
    if "warm" not in _cached:
        # first call: run once so the NEFF is loaded on every core before
        # the "real" execution (cold NEFF loads stagger core start times
        # and inflate cross-core sync waits)
        _exec_once(dev_in)
        _cached["warm"] = True
    res = _collect(_exec_once(dev_in))
    return _postprocess(res)
